# revision 15
# baseline (speedup 1.0000x reference)
"""DeepRetrieval beam-search kernel for 8 Trainium2 NeuronCores.

Strategy: data-parallel over the batch (B=512 -> 64 users per core).
Each core runs its users' full 3-level beam tree:
  level l: logits = relu(x @ W1 + b1) [BN folded into W2] @ W2' -> softmax
  top-5 per row via DVE max/max_index over exp(logits), beam expansion
  via indirect-DMA gathers from node_emb, final 125-path combine on-device.
"""

import numpy as np

import concourse.bass as bass
import concourse.bacc as bacc
import concourse.mybir as mybir
import concourse.tile as tile
from concourse.bass import IndirectOffsetOnAxis
from concourse.bass_utils import run_bass_kernel_spmd
from concourse.masks import make_identity

F32 = mybir.dt.float32
F32R = mybir.dt.float32r
U32 = mybir.dt.uint32

NCORES = 8
BC = 64          # users per core
K = 5            # beam size
D = 128          # embedding dim
HID = 256        # mlp hidden
N3 = 10000       # logits per level
NTW = 512        # logit tile width
# 20 logit tiles: 19x512 + 272
NTILES = [(t * NTW, NTW) for t in range(19)] + [(19 * NTW, N3 - 19 * NTW)]
SPLIT_T = 10                 # tiles [0,10) -> E-half A, [10,20) -> half B
WA = SPLIT_T * NTW           # 5120
WB = N3 - WA                 # 4880

LEVELS = [
    # (d_in_chunks, rows)
    (1, BC),             # level 1: x = u [64, 128]
    (2, BC * K),         # level 2: x = [u, e1] [320, 256]
    (3, BC * K * K),     # level 3: x = [u, e1, e2] [1600, 384]
]


def _row_tiles(rows):
    out = []
    s = 0
    while s < rows:
        r = min(128, rows - s)
        out.append((s, r))
        s += r
    return out


WQ = [(0, 2560), (2560, 2560), (5120, 2560), (7680, 2320)]  # quarter cols


def _load_w2(nc, pool, w2_dram, lvl):
    """W2 as 8 separate quarter tiles -> fine-grained DMA deps."""
    tiles = []
    for c in range(2):
        row = []
        for q, (lo, w) in enumerate(WQ):
            t = pool.tile([128, w], F32R, tag=f"w2_c{c}q{q}",
                          name=f"w2_{lvl}_c{c}q{q}")
            nc.sync.dma_start(t[:], w2_dram[:, c * N3 + lo:c * N3 + lo + w])
            row.append(t)
        tiles.append(row)
    return tiles


def _w2_slice(tiles, c, c0, w):
    """AP for cols [c0, c0+w) of hidden-chunk c from quarter tiles."""
    for q, (lo, wq) in enumerate(WQ):
        if lo <= c0 and c0 + w <= lo + wq:
            return tiles[c][q][:, c0 - lo:c0 - lo + w]
    raise AssertionError(f"slice {c0}+{w} straddles quarters")


def _set_dyn_queue(inst, i):
    q = i % 4
    if q:
        inst.ins.queue = f"qPoolDynamic{q}"


def _build_program(has_b2):
    nc = bacc.Bacc("TRN2", target_bir_lowering=False, debug=False,
                   num_swdge_queues=4)

    # ---- external I/O ----
    uT = nc.dram_tensor("uT", [D, BC], F32R, kind="ExternalInput").ap()
    w1 = [
        nc.dram_tensor(f"w1_{l}", [128, nd * HID], F32R, kind="ExternalInput").ap()
        for l, (nd, _) in enumerate(LEVELS)
    ]
    b1 = [
        nc.dram_tensor(f"b1_{l}", [128, 2], F32, kind="ExternalInput").ap()
        for l in range(3)
    ]
    w2 = [
        nc.dram_tensor(f"w2_{l}", [128, 2 * N3], F32R, kind="ExternalInput").ap()
        for l in range(3)
    ]
    b2 = None
    if has_b2:
        b2 = [
            nc.dram_tensor(f"b2_{l}", [1, N3], F32R, kind="ExternalInput").ap()
            for l in range(3)
        ]
    node1 = nc.dram_tensor("node1", [N3, D], F32, kind="ExternalInput").ap()
    node2 = nc.dram_tensor("node2", [N3, D], F32, kind="ExternalInput").ap()

    paths_out = nc.dram_tensor("paths_f", [BC, 15], F32, kind="ExternalOutput").ap()
    probs_out = nc.dram_tensor("probs", [BC, K], F32, kind="ExternalOutput").ap()

    # ---- internal DRAM scratch ----
    i1rep = nc.dram_tensor("i1rep", [BC * K, 1], U32).ap()          # [320]
    i2flat = nc.dram_tensor("i2flat", [BC * K * K, 1], U32).ap()    # [1600]
    p2flat = nc.dram_tensor("p2flat", [BC * K * K, 1], F32).ap()    # [1600]
    p3flat = nc.dram_tensor("p3flat", [BC * K * K * K, 1], F32).ap()  # [8000]
    i3flat = nc.dram_tensor("i3flat", [BC * K * K * K, 1], F32).ap()  # [8000]

    with tile.TileContext(nc) as tc:
        cst = tc.alloc_tile_pool(name="cst", bufs=1)
        w2p = tc.alloc_tile_pool(name="w2p", bufs=1)
        xp = tc.alloc_tile_pool(name="xp", bufs=1)
        ep = tc.alloc_tile_pool(name="ep", bufs=3)
        hp = tc.alloc_tile_pool(name="hp", bufs=4)
        sm = tc.alloc_tile_pool(name="sm", bufs=3)
        gp = tc.alloc_tile_pool(name="gp", bufs=3)
        pp = tc.alloc_tile_pool(name="pp", bufs=1)
        ps_l = tc.alloc_tile_pool(name="ps_l", bufs=5, space="PSUM")
        ps_h = tc.alloc_tile_pool(name="ps_h", bufs=1, space="PSUM")
        ps_t = tc.alloc_tile_pool(name="ps_t", bufs=2, space="PSUM")

        # ---- constants in SBUF ----
        uT_sb = cst.tile([D, BC], F32R, tag="uT")
        nc.sync.dma_start(uT_sb[:], uT[:])
        w1_sb = [cst.tile([128, nd * HID], F32R, tag=f"w1_{l}", name=f"w1sb_{l}")
                 for l, (nd, _) in enumerate(LEVELS)]
        for l in range(3):
            nc.sync.dma_start(w1_sb[l][:], w1[l][:])
        b1_sb = [cst.tile([128, 2], F32, tag=f"b1_{l}", name=f"b1sb_{l}") for l in range(3)]
        for l in range(3):
            nc.sync.dma_start(b1_sb[l][:], b1[l][:])
        ident = cst.tile([128, 128], F32, tag="ident")
        make_identity(nc, ident[:])
        ones_sb = None
        b2_sb = None
        if has_b2:
            ones_sb = cst.tile([1, 128], F32R, tag="ones")
            nc.vector.memset(ones_sb[:], 1.0)

        # X2T: two chunks [128, 320]; X3T: three chunks [128, 1600]
        x2t = [xp.tile([128, BC * K], F32R, tag=f"x2t_{c}", name=f"x2t_{c}") for c in range(2)]
        x3t = [xp.tile([128, BC * K * K], F32R, tag=f"x3t_{c}", name=f"x3t_{c}") for c in range(3)]

        def level(lvl, xt_chunks, w2_sb, b2_t):
            """Runs one level; returns (p8_all, gidxf_all, rts).

            Per row-tile only 3 max + 2 max_index run on DVE; all small ops
            are batched per level to keep the DVE queue unblocked.
            """
            nd, rows = LEVELS[lvl]
            rts = _row_tiles(rows)
            nrt = len(rts)
            vm8_all = pp.tile([128, nrt * 8], F32, tag=f"vm8_{lvl}",
                              name=f"vm8_{lvl}")
            via_all = pp.tile([128, nrt * 8], U32, tag=f"via_{lvl}",
                              name=f"via_{lvl}")
            vib_all = pp.tile([128, nrt * 8], U32, tag=f"vib_{lvl}",
                              name=f"vib_{lvl}")
            zp_all = pp.tile([128, nrt * len(NTILES)], F32, tag=f"zp_{lvl}",
                             name=f"zp_{lvl}")
            if rts[-1][1] < 128:
                # pad rows of the last row-tile stay unwritten; init them so
                # the batched epilogue reads defined values
                nc.gpsimd.memset(zp_all[:, :], 1.0)
                nc.gpsimd.memset(vm8_all[:, :], 0.0)
                nc.gpsimd.memset(via_all[:, :], 0)
                nc.gpsimd.memset(vib_all[:, :], 0)
            for rt, (s, R) in enumerate(rts):
                # H^T = relu(W1^T @ X^T + b1): two hidden chunks [128, R]
                h_sb = []
                for hc in range(2):
                    psh = ps_h.tile([128, 128], F32, tag="psh")
                    for dc in range(nd):
                        nc.tensor.matmul(
                            psh[:, :R],
                            lhsT=w1_sb[lvl][:, dc * HID + hc * 128:
                                            dc * HID + hc * 128 + 128],
                            rhs=xt_chunks[dc][:, s:s + R],
                            start=(dc == 0),
                            stop=(dc == nd - 1),
                        )
                    ht = hp.tile([128, 128], F32R, tag="h")
                    nc.scalar.activation(
                        out=ht[:, :R], in_=psh[:, :R],
                        func=mybir.ActivationFunctionType.Relu,
                        bias=b1_sb[lvl][:, hc:hc + 1],
                    )
                    h_sb.append(ht)

                # logits -> exp -> E halves, Z partials
                ea = ep.tile([128, WA], F32, tag="E")
                eb = ep.tile([128, WB], F32, tag="E")
                for t, (c0, w) in enumerate(NTILES):
                    psl = ps_l.tile([128, NTW], F32, tag="psl")
                    nc.tensor.matmul(
                        psl[:R, :w], lhsT=h_sb[0][:, :R],
                        rhs=_w2_slice(w2_sb, 0, c0, w),
                        start=True, stop=False,
                    )
                    nc.tensor.matmul(
                        psl[:R, :w], lhsT=h_sb[1][:, :R],
                        rhs=_w2_slice(w2_sb, 1, c0, w),
                        start=False, stop=not has_b2,
                    )
                    if has_b2:
                        nc.tensor.matmul(
                            psl[:R, :w], lhsT=ones_sb[:, :R],
                            rhs=b2_t[:, c0:c0 + w],
                            start=False, stop=True,
                        )
                    if t < SPLIT_T:
                        dst = ea[:R, c0:c0 + w]
                    else:
                        dst = eb[:R, c0 - WA:c0 - WA + w]
                    nc.scalar.activation(
                        out=dst, in_=psl[:R, :w],
                        func=mybir.ActivationFunctionType.Exp,
                        accum_out=zp_all[:R, rt * len(NTILES) + t:
                                         rt * len(NTILES) + t + 1],
                    )

                # top-8 values per half, then merged top-8 + per-half indices
                cvals = sm.tile([128, 16], F32, tag="cvals")
                nc.vector.max(out=cvals[:R, 0:8], in_=ea[:R, :])
                nc.vector.max(out=cvals[:R, 8:16], in_=eb[:R, :])
                nc.vector.max(out=vm8_all[:R, rt * 8:rt * 8 + 8],
                              in_=cvals[:R, :])
                nc.vector.max_index(via_all[:R, rt * 8:rt * 8 + 8],
                                    vm8_all[:R, rt * 8:rt * 8 + 8], ea[:R, :])
                nc.vector.max_index(vib_all[:R, rt * 8:rt * 8 + 8],
                                    vm8_all[:R, rt * 8:rt * 8 + 8], eb[:R, :])

            # ---- batched level epilogue ----
            W8 = nrt * 8
            zs = pp.tile([128, nrt], F32, tag=f"zs_{lvl}", name=f"zs_{lvl}")
            nc.vector.tensor_reduce(
                zs[:, :],
                zp_all[:, :].rearrange("p (r t) -> p r t", r=nrt),
                axis=mybir.AxisListType.X, op=mybir.AluOpType.add,
            )
            zi = pp.tile([128, nrt], F32, tag=f"zi_{lvl}", name=f"zi_{lvl}")
            nc.vector.reciprocal(zi[:, :], zs[:, :])
            p8_all = pp.tile([128, W8], F32, tag=f"p8_{lvl}",
                             name=f"p8_{lvl}")
            nc.vector.tensor_tensor(
                out=p8_all[:, :].rearrange("p (r e) -> p r e", r=nrt),
                in0=vm8_all[:, :].rearrange("p (r e) -> p r e", r=nrt),
                in1=zi[:, :].rearrange("p (r o) -> p r o", o=1)
                            .to_broadcast([128, nrt, 8]),
                op=mybir.AluOpType.mult,
            )
            viaf = pp.tile([128, W8], F32, tag=f"viaf_{lvl}",
                           name=f"viaf_{lvl}")
            vibf = pp.tile([128, W8], F32, tag=f"vibf_{lvl}",
                           name=f"vibf_{lvl}")
            nc.vector.tensor_copy(viaf[:, :], via_all[:, :])
            nc.vector.tensor_copy(vibf[:, :], vib_all[:, :])
            # idxB' = min(idxB + WA, N3-1); unmatched (2^32-1) clamps harmless
            nc.vector.tensor_scalar(
                vibf[:, :], vibf[:, :], float(WA), float(N3 - 1),
                op0=mybir.AluOpType.add, op1=mybir.AluOpType.min,
            )
            gidxf = pp.tile([128, W8], F32, tag=f"gidxf_{lvl}",
                            name=f"gidxf_{lvl}")
            nc.vector.tensor_tensor(
                out=gidxf[:, :], in0=viaf[:, :], in1=vibf[:, :],
                op=mybir.AluOpType.min,
            )
            return p8_all, gidxf, rts

        # ---------- level 1 ----------
        w2_sb = _load_w2(nc, w2p, w2[0], 0)
        b2_t = None
        if has_b2:
            b2_t = pp.tile([1, N3], F32R, tag="b2")
            nc.sync.dma_start(b2_t[:], b2[0][:])

        p1_sb, i1f_sb, _ = level(0, [uT_sb], w2_sb, b2_t)
        gidx_u = pp.tile([BC, 8], U32, tag="i1u")
        nc.vector.tensor_copy(gidx_u[:, :], i1f_sb[:BC, :])
        # i1rep[u*5+j] = i1[u, j]
        nc.sync.dma_start(
            i1rep.rearrange("(u j) one -> u (j one)", j=K),
            gidx_u[:, :K],
        )

        # ---------- gather e1 = node1[i1] and build X2T ----------
        # X2T chunk 0: u broadcast over beams
        nc.scalar.copy(
            out=x2t[0][:, :],
            in_=uT_sb[:].to_broadcast([D, BC, K]),
        )
        for gt, (s, R) in enumerate(_row_tiles(BC * K)):
            it = gp.tile([128, 1], U32, tag="git")
            nc.sync.dma_start(it[:R, :], i1rep[s:s + R, :])
            g = gp.tile([128, D], F32, tag="g")
            gi = nc.gpsimd.indirect_dma_start(
                out=g[:R, :], out_offset=None,
                in_=node1[:, :],
                in_offset=IndirectOffsetOnAxis(ap=it[:R, :1], axis=0),
            )
            _set_dyn_queue(gi, gt)
            pst = ps_t.tile([128, 128], F32, tag="pst")
            nc.tensor.transpose(pst[:D, :R], g[:R, :], ident[:R, :R])
            nc.scalar.copy(out=x2t[1][:, s:s + R], in_=pst[:D, :R])

        # ---------- level 2 ----------
        w2_sb2 = _load_w2(nc, w2p, w2[1], 1)
        if has_b2:
            b2_t = pp.tile([1, N3], F32R, tag="b2")
            nc.sync.dma_start(b2_t[:], b2[1][:])

        p8_l2, gidxf_l2, rts_l2 = level(1, x2t, w2_sb2, b2_t)
        gu_l2 = pp.tile([128, len(rts_l2) * 8], U32, tag="gu_l2")
        nc.vector.tensor_copy(gu_l2[:, :], gidxf_l2[:, :])
        for rt, (s, R) in enumerate(rts_l2):
            nc.sync.dma_start(
                i2flat[s * K:(s + R) * K, :]
                    .rearrange("(r j) one -> r (j one)", j=K),
                gu_l2[:R, rt * 8:rt * 8 + K],
            )
            nc.sync.dma_start(
                p2flat[s * K:(s + R) * K, :]
                    .rearrange("(r j) one -> r (j one)", j=K),
                p8_l2[:R, rt * 8:rt * 8 + K],
            )

        # ---------- gather e2 = node2[i2] and build X3T ----------
        nc.scalar.copy(
            out=x3t[0][:, :],
            in_=uT_sb[:].to_broadcast([D, BC, K * K]),
        )
        nc.scalar.copy(
            out=x3t[1][:, :],
            in_=x2t[1][:].to_broadcast([D, BC * K, K]),
        )
        for gt, (s, R) in enumerate(_row_tiles(BC * K * K)):
            it = gp.tile([128, 1], U32, tag="git")
            nc.sync.dma_start(it[:R, :], i2flat[s:s + R, :])
            g = gp.tile([128, D], F32, tag="g")
            gi = nc.gpsimd.indirect_dma_start(
                out=g[:R, :], out_offset=None,
                in_=node2[:, :],
                in_offset=IndirectOffsetOnAxis(ap=it[:R, :1], axis=0),
            )
            _set_dyn_queue(gi, gt)
            pst = ps_t.tile([128, 128], F32, tag="pst")
            nc.tensor.transpose(pst[:D, :R], g[:R, :], ident[:R, :R])
            nc.scalar.copy(out=x3t[2][:, s:s + R], in_=pst[:D, :R])

        # ---------- level 3 ----------
        w2_sb3 = _load_w2(nc, w2p, w2[2], 2)
        if has_b2:
            b2_t = pp.tile([1, N3], F32R, tag="b2")
            nc.sync.dma_start(b2_t[:], b2[2][:])

        p8_l3, gidxf_l3, rts_l3 = level(2, x3t, w2_sb3, b2_t)
        for rt, (s, R) in enumerate(rts_l3):
            nc.sync.dma_start(
                p3flat[s * K:(s + R) * K, :]
                    .rearrange("(r j) one -> r (j one)", j=K),
                p8_l3[:R, rt * 8:rt * 8 + K],
            )
            nc.sync.dma_start(
                i3flat[s * K:(s + R) * K, :]
                    .rearrange("(r j) one -> r (j one)", j=K),
                gidxf_l3[:R, rt * 8:rt * 8 + K],
            )

        # ---------- combine: top-5 of 125 path probs per user ----------
        p2u = pp.tile([BC, K * K], F32, tag="p2u")
        nc.sync.dma_start(
            p2u[:, :],
            p2flat[:, :].rearrange("(u q) one -> u (q one)", u=BC),
        )
        i2u = pp.tile([BC, K * K], U32, tag="i2u")
        nc.sync.dma_start(
            i2u[:, :],
            i2flat[:, :].rearrange("(u q) one -> u (q one)", u=BC),
        )
        i2uf = pp.tile([BC, K * K], F32, tag="i2uf")
        nc.vector.tensor_copy(i2uf[:, :], i2u[:, :])
        p3u = pp.tile([BC, K * K * K], F32, tag="p3u")
        nc.sync.dma_start(
            p3u[:, :],
            p3flat[:, :].rearrange("(u q) one -> u (q one)", u=BC),
        )
        i3u = pp.tile([BC, K * K * K], F32, tag="i3u")
        nc.sync.dma_start(
            i3u[:, :],
            i3flat[:, :].rearrange("(u q) one -> u (q one)", u=BC),
        )

        # p12[u, i*5+j] = p1[u,i] * p2[u, i*5+j]
        p12 = pp.tile([BC, K * K], F32, tag="p12")
        nc.vector.tensor_tensor(
            out=p12[:, :],
            in0=p1_sb[:BC, :K].to_broadcast([BC, K, K]),
            in1=p2u[:, :],
            op=mybir.AluOpType.mult,
        )
        ppaths = pp.tile([BC, K * K * K], F32, tag="ppaths")
        nc.vector.tensor_tensor(
            out=ppaths[:, :],
            in0=p12[:, :].to_broadcast([BC, K * K, K]),
            in1=p3u[:, :],
            op=mybir.AluOpType.mult,
        )

        ppv = pp.tile([BC, 8], F32, tag="ppv")
        nc.vector.max(out=ppv[:, :], in_=ppaths[:, :])
        pos = pp.tile([BC, 8], U32, tag="pos")
        nc.vector.max_index(pos[:, :], ppv[:, :], ppaths[:, :])

        # decode: one-hot masks over 125 positions for the 5 winning slots
        iota = pp.tile([BC, K * 125], U32, tag="iota")
        nc.gpsimd.iota(iota[:, :], pattern=[[0, K], [1, 125]],
                       base=0, channel_multiplier=0)
        mask = pp.tile([BC, K * 125], F32, tag="mask")
        nc.vector.tensor_tensor(
            out=mask[:, :],
            in0=iota[:, :],
            in1=pos[:, :K].to_broadcast([BC, K, 125]),
            op=mybir.AluOpType.is_equal,
        )

        nsel = pp.tile([BC, 15], F32, tag="nsel")
        prod = pp.tile([BC, K * 125], F32, tag="prod")
        arr_aps = [
            i1f_sb[:BC, :K].rearrange("u (s i) -> u s i", s=1)
                         .to_broadcast([BC, K, K]).to_broadcast([BC, K, K, 25]),
            i2uf[:, :].rearrange("u (s q) -> u s q", s=1)
                      .to_broadcast([BC, K, K * K])
                      .to_broadcast([BC, K, K * K, K]),
            i3u[:, :].rearrange("u (s q) -> u s q", s=1)
                     .to_broadcast([BC, K, 125]),
        ]
        for a in range(3):
            nc.vector.tensor_tensor(
                out=prod[:, :], in0=mask[:, :], in1=arr_aps[a],
                op=mybir.AluOpType.mult,
            )
            nc.vector.tensor_reduce(
                nsel[:, a * K:(a + 1) * K],
                prod[:, :].rearrange("u (s q) -> u s q", s=K),
                axis=mybir.AxisListType.X,
                op=mybir.AluOpType.add,
            )

        nc.sync.dma_start(paths_out[:, :], nsel[:, :])
        nc.sync.dma_start(probs_out[:, :], ppv[:, :K])

        for p in reversed((cst, w2p, xp, ep, hp, sm, gp, pp, ps_l, ps_h, ps_t)):
            p.release()

    nc.compile()
    return nc


_PROG_CACHE = {}


def _get_program(has_b2):
    key = bool(has_b2)
    if key not in _PROG_CACHE:
        _PROG_CACHE[key] = _build_program(key)
    return _PROG_CACHE[key]


def _prep_inputs(params, user_idx):
    """Fold BN into W2/b2 (in float64) and lay out per-core input maps."""
    def npf(x):
        return np.asarray(x)

    user_emb = npf(params['user_emb'])
    node_emb = npf(params['node_emb'])
    u = user_emb[np.asarray(user_idx)]          # [512, 128]

    lvl_tensors = []
    has_b2 = False
    for name in ('l1', 'l2', 'l3'):
        lp = params[name]
        W1 = npf(lp['W1']).astype(np.float64)
        b1v = npf(lp['b1']).astype(np.float64)
        gamma = npf(lp['gamma']).astype(np.float64)
        beta = npf(lp['beta']).astype(np.float64)
        mean = npf(lp['mean']).astype(np.float64)
        var = npf(lp['var']).astype(np.float64)
        W2 = npf(lp['W2']).astype(np.float64)
        b2v = npf(lp['b2']).astype(np.float64)

        s = gamma / np.sqrt(var + 1e-5)
        t = beta - mean * s
        W2p = (s[:, None] * W2).astype(np.float32)          # [256, 10000]
        b2p = (b2v + t @ W2).astype(np.float32)             # [10000]
        if np.any(b2p != 0):
            has_b2 = True

        d_in = W1.shape[0]
        nd = d_in // 128
        # w1 layout [128, nd*256]: block dc -> W1[128dc:128dc+128, :]
        w1_l = np.concatenate(
            [W1[dc * 128:(dc + 1) * 128, :] for dc in range(nd)], axis=1
        ).astype(np.float32)
        # b1 layout [128, 2]
        b1_l = np.stack([b1v[0:128], b1v[128:256]], axis=1).astype(np.float32)
        # w2 layout [128, 2*N3]: chunk c -> W2p[128c:128c+128, :]
        w2_l = np.concatenate([W2p[0:128, :], W2p[128:256, :]],
                              axis=1).astype(np.float32)
        lvl_tensors.append((w1_l, b1_l, w2_l, b2p[None, :]))

    node1 = np.ascontiguousarray(node_emb[0:N3]).astype(np.float32)
    node2 = np.ascontiguousarray(node_emb[N3:2 * N3]).astype(np.float32)

    in_maps = []
    for c in range(NCORES):
        uc = u[c * BC:(c + 1) * BC]
        m = {
            'uT': np.ascontiguousarray(uc.T).astype(np.float32),
            'node1': node1,
            'node2': node2,
        }
        for l in range(3):
            w1_l, b1_l, w2_l, b2_l = lvl_tensors[l]
            m[f'w1_{l}'] = w1_l
            m[f'b1_{l}'] = b1_l
            m[f'w2_{l}'] = w2_l
            if has_b2:
                m[f'b2_{l}'] = b2_l
        in_maps.append(m)
    return in_maps, has_b2


def _assemble(results):
    paths = np.empty((NCORES * BC, K, 3), dtype=np.int32)
    probs = np.empty((NCORES * BC, K), dtype=np.float32)
    for c, r in enumerate(results):
        pf = r['paths_f'].reshape(BC, 3, K).transpose(0, 2, 1)  # [64, 5, 3]
        n = np.rint(pf).astype(np.int64)
        n[:, :, 1] += N3
        n[:, :, 2] += 2 * N3
        paths[c * BC:(c + 1) * BC] = n.astype(np.int32)
        probs[c * BC:(c + 1) * BC] = r['probs']
    return paths, probs


def kernel(params, user_idx, beam_size, _trace=False):
    assert int(beam_size) == K
    in_maps, has_b2 = _prep_inputs(params, user_idx)
    nc = _get_program(has_b2)
    res = run_bass_kernel_spmd(nc, in_maps, list(range(NCORES)),
                               trace=_trace)
    paths, probs = _assemble(res.results)
    if _trace:
        kernel._last_exec_time_ns = res.exec_time_ns
        kernel._last_profile = res
    return paths, probs
